# revision 16
# baseline (speedup 1.0000x reference)
"""Distributed Llama-attention Bass kernel for 8 TRN2 NeuronCores.

Sharding: tensor-parallel over heads (core c owns query heads 4c..4c+3 and
KV head c), per-chunk AllGathers of attention outputs (bf16) pipelined
against later chunks, and a column-shard of wo so each core produces a
disjoint [2048, 512] column slice of the final output (no all-reduce).

v4: host supplies hsT (pre-transposed, bf16) and bf16 weights (no device
transposes/conversions); hsT streams from DRAM in 512KB batches over three
projection passes (k+v first so rope-k clears before attention); causal
diagonal tiles use partial-width matmuls; the softmax row-sum/normalize
epilogue runs entirely on gpsimd (partition_all_reduce) + DVE, keeping the
PE stream pure matmul; oproj loads gathered heads per-head and accumulates
h-major so only the final AllGather is exposed.
"""

import math
import sys

import numpy as np

sys.path.insert(0, "/opt/trn_rl_repo")

import ml_dtypes  # noqa: E402

import concourse.bass as bass  # noqa: E402
import concourse.bass_isa as bass_isa  # noqa: E402
import concourse.mybir as mybir  # noqa: E402
import concourse.tile as tile  # noqa: E402
from concourse import bacc  # noqa: E402
from concourse.bass_utils import run_bass_kernel_spmd  # noqa: E402
from concourse.masks import make_identity  # noqa: E402

F32 = mybir.dt.float32
BF16 = mybir.dt.bfloat16
Alu = mybir.AluOpType
Act = mybir.ActivationFunctionType

NCORES = 8
S = 2048
D = 4096
H = 32
HKV = 8
HD = 128
NH = H // NCORES          # 4 local query heads
QCOLS = NH * HD           # 512 local q-proj cols
CHUNK = 512               # s-chunk size
NCHUNK = S // CHUNK       # 4
DC = D // 128             # 32 d-chunks
SCALE = 1.0 / math.sqrt(HD)

_CACHED = {}


def _build_graph():
    nc = bacc.Bacc(
        "TRN2",
        target_bir_lowering=False,
        debug=False,
        num_devices=NCORES,
    )

    hsT_d = nc.dram_tensor("hsT", [D, S], BF16, kind="ExternalInput").ap()
    wq_d = nc.dram_tensor("wq", [D, QCOLS], BF16, kind="ExternalInput").ap()
    wk_d = nc.dram_tensor("wk", [D, HD], BF16, kind="ExternalInput").ap()
    wv_d = nc.dram_tensor("wv", [D, HD], BF16, kind="ExternalInput").ap()
    wo_d = nc.dram_tensor("wo", [D, QCOLS], BF16, kind="ExternalInput").ap()
    cos_d = nc.dram_tensor("cos", [HD, S], BF16, kind="ExternalInput").ap()
    sin_d = nc.dram_tensor("sin", [HD, S], BF16, kind="ExternalInput").ap()
    out_d = nc.dram_tensor("out", [S, QCOLS], F32, kind="ExternalOutput").ap()

    hsT_v = hsT_d.rearrange("(i p) s -> p i s", p=128)
    wq_v = wq_d.rearrange("(i p) w -> p i w", p=128)
    wk_v = wk_d.rearrange("(i p) w -> p i w", p=128)
    wv_v = wv_d.rearrange("(i p) w -> p i w", p=128)
    wo_v = wo_d.rearrange("(i p) w -> p i w", p=128)

    with tile.TileContext(nc) as tc:
        with (
            tc.tile_pool(name="persist", bufs=1) as pp,
            tc.tile_pool(name="hsd", bufs=3) as hsdp,
            tc.tile_pool(name="qtp", bufs=2) as qtp,
            tc.tile_pool(name="otp", bufs=4) as otp,
            tc.tile_pool(name="ep", bufs=4) as ep,
            tc.tile_pool(name="rt", bufs=2) as rtp,
            # prj: 2 proj accumulators; sc: scores + v-transpose; acc: AV + oproj
            tc.tile_pool(name="ps_prj", bufs=2, space="PSUM") as ps_prj,
            tc.tile_pool(name="ps_sc", bufs=2, space="PSUM") as ps_sc,
            tc.tile_pool(name="ps_acc", bufs=4, space="PSUM") as ps_acc,
            tc.tile_pool(name="dram", bufs=1, space="DRAM") as dram,
        ):
            # ---------------- persistent SBUF tensors ----------------
            wq_bf = pp.tile([128, DC, QCOLS], BF16, tag="wq")
            wk_bf = pp.tile([128, DC, HD], BF16, tag="wk")
            wv_bf = pp.tile([128, DC, HD], BF16, tag="wv")
            wo_bf = pp.tile([128, DC, QCOLS], BF16, tag="wo")
            cos_sb = pp.tile([HD, S], BF16, tag="cos")
            sin_sb = pp.tile([HD, S], BF16, tag="sin")
            kT_bf = pp.tile([HD, S], BF16, tag="kt")
            v_bf = pp.tile([128, S // 128, HD], BF16, tag="v")
            ident = pp.tile([128, 128], BF16, tag="id")
            ones_col = pp.tile([128, 1], BF16, tag="onc")
            ones_row = pp.tile([1, 128], BF16, tag="onr")

            attn_in = [
                [
                    dram.tile(
                        [HD, CHUNK], BF16, tag=f"ain{j}_{h}", name=f"ain{j}_{h}"
                    )
                    for h in range(NH)
                ]
                for j in range(NCHUNK)
            ]
            attn_all = [
                [
                    dram.tile(
                        [NCORES * HD, CHUNK], BF16, tag=f"aall{j}_{h}",
                        addr_space="Shared", name=f"aall{j}_{h}",
                    )
                    for h in range(NH)
                ]
                for j in range(NCHUNK)
            ]

            # ---------------- constants ----------------
            make_identity(nc, ident[:])
            nc.gpsimd.memset(ones_col[:], 1.0)
            nc.gpsimd.memset(ones_row[:], 1.0)
            # cos/sin on the gpsimd queue so they don't delay weight/hs DMA
            nc.gpsimd.dma_start(out=cos_sb[:], in_=cos_d)
            nc.gpsimd.dma_start(out=sin_sb[:], in_=sin_d)

            def load_weight(dst_bf, src_v, n=4):
                for i in range(0, DC, n):
                    nc.scalar.dma_start(
                        out=dst_bf[:, i : i + n, :], in_=src_v[:, i : i + n, :]
                    )

            def emit_epilogues(j, pairs):
                for h, pso, racc in pairs:
                    # partition reduce 128 -> 1 with a single ones-matmul
                    # (racc is bf16 and feeds the matmul directly)
                    psr = ps_sc.tile([1, CHUNK], F32, tag="sc")
                    nc.tensor.matmul(
                        psr[:], lhsT=ones_col[:], rhs=racc[:],
                        start=True, stop=True,
                    )
                    rc = ep.tile([1, CHUNK], BF16, tag="rc", bufs=2)
                    with nc.allow_low_precision(
                        reason="1/rowsum bcast; bf16 fine for softmax norm"
                    ):
                        nc.vector.reciprocal(rc[:], psr[:])
                    psb = ps_sc.tile([128, CHUNK], F32, tag="sc")
                    nc.tensor.matmul(
                        psb[:], lhsT=ones_row[:], rhs=rc[:],
                        start=True, stop=True,
                    )
                    sbb = ep.tile([128, CHUNK], F32, tag="os", bufs=3)
                    nc.scalar.copy(out=sbb[:], in_=psb[:])
                    ao = ep.tile([128, CHUNK], BF16, tag="ao", bufs=2)
                    nc.vector.tensor_tensor(
                        out=ao[:], in0=pso[:], in1=sbb[:], op=Alu.mult
                    )
                    # ao goes out on the gpsimd queue: the sync queue carries
                    # the hsT stream and must not block behind the epilogue
                    nc.gpsimd.dma_start(out=attn_in[j][h][:, :], in_=ao[:])
                    nc.gpsimd.collective_compute(
                        "AllGather",
                        Alu.bypass,
                        replica_groups=[list(range(NCORES))],
                        ins=[attn_in[j][h].opt()],
                        outs=[attn_all[j][h].opt()],
                    )

            def rope(psrc, dst_ap, sl):
                t1 = rtp.tile([128, CHUNK], BF16, tag="rt")
                t2 = rtp.tile([128, CHUNK], BF16, tag="rt")
                nc.vector.tensor_tensor(
                    out=t1[0:64, :], in0=psrc[64:128, :],
                    in1=sin_sb[0:64, sl], op=Alu.mult,
                )
                nc.vector.tensor_tensor(
                    out=t1[64:128, :], in0=psrc[0:64, :],
                    in1=sin_sb[64:128, sl], op=Alu.mult,
                )
                nc.vector.tensor_tensor(
                    out=t2[:], in0=psrc[:], in1=cos_sb[:, sl], op=Alu.mult
                )
                nc.vector.tensor_tensor(
                    out=dst_ap, in0=t1[:], in1=t2[:], op=Alu.add
                )

            def proj_pass(j, accs, weights):
                """One streamed pass over hsT d-chunks for s-chunk j (512KB
                DMA batches to amortize the ~0.6us per-DMA trigger cost)."""
                s0 = j * CHUNK
                NB = 4
                for ib in range(0, DC, NB):
                    hsd = hsdp.tile([128, NB, CHUNK], BF16, tag="hsd")
                    nc.sync.dma_start(
                        out=hsd[:], in_=hsT_v[:, ib : ib + NB, s0 : s0 + CHUNK]
                    )
                    for k in range(NB):
                        i = ib + k
                        st, sp = (i == 0), (i == DC - 1)
                        for acc, wsel in zip(accs, weights):
                            nc.tensor.matmul(
                                acc[:], lhsT=wsel(i), rhs=hsd[:, k, :],
                                start=st, stop=sp,
                            )

            def projections(j, qT, pend):
                """Three 2-wide passes: (k,v) first so rope-k/v are ready
                well before attention, then (q0,q1) in the sc ring and
                (q2,q3) in the prj ring — alternating rings so no pass
                WARs the previous pass's in-flight rope.  pend (a deferred
                attention epilogue) resolves behind pass1's stream."""
                s0 = j * CHUNK
                sl = bass.ds(s0, CHUNK)

                psk = ps_prj.tile([128, CHUNK], F32, tag="p")
                psv = ps_prj.tile([128, CHUNK], F32, tag="p")
                proj_pass(
                    j, [psk, psv],
                    [lambda i: wk_bf[:, i, :], lambda i: wv_bf[:, i, :]],
                )
                rope(psk[:], kT_bf[:, sl], sl)
                vT_sb = ep.tile([128, CHUNK], BF16, tag="vts", bufs=2)
                nc.scalar.copy(out=vT_sb[:], in_=psv[:])
                if pend is not None:
                    emit_epilogues(*pend)

                psq0 = ps_sc.tile([128, CHUNK], F32, tag="sc")
                psq1 = ps_sc.tile([128, CHUNK], F32, tag="sc")
                proj_pass(
                    j, [psq0, psq1],
                    [
                        lambda i: wq_bf[:, i, 0:HD],
                        lambda i: wq_bf[:, i, HD : 2 * HD],
                    ],
                )
                rope(psq0[:], qT[:, 0, :], sl)
                rope(psq1[:], qT[:, 1, :], sl)

                psq2 = ps_prj.tile([128, CHUNK], F32, tag="p")
                psq3 = ps_prj.tile([128, CHUNK], F32, tag="p")
                proj_pass(
                    j, [psq2, psq3],
                    [
                        lambda i: wq_bf[:, i, 2 * HD : 3 * HD],
                        lambda i: wq_bf[:, i, 3 * HD : 4 * HD],
                    ],
                )
                rope(psq2[:], qT[:, 2, :], sl)
                rope(psq3[:], qT[:, 3, :], sl)
                # v computed as vT [hd, s]; PE-transpose back to [s, hd].
                # Emitted after pass3 so its sc-ring slot's rope is done.
                psv2 = ps_sc.tile([128, 4, 128], BF16, tag="sc")
                for ss in range(CHUNK // 128):
                    nc.tensor.transpose(
                        psv2[:, ss, :],
                        vT_sb[:, 128 * ss : 128 * (ss + 1)],
                        ident[:],
                    )
                nc.vector.tensor_copy(
                    out=v_bf[:, (CHUNK // 128) * j : (CHUNK // 128) * (j + 1), :],
                    in_=psv2[:],
                )

            def attention(j, qT):
                nk = 4 * (j + 1)  # causal: key tiles 0..nk-1

                def score_block(h, kcs):
                    es = []
                    for kc in kcs:
                        t = kc - 4 * j  # >= 0 on diagonal tiles
                        off = 128 * t if t > 0 else 0
                        pss = ps_sc.tile([128, CHUNK], F32, tag="sc")
                        nc.tensor.matmul(
                            pss[:, off:CHUNK],
                            lhsT=kT_bf[:, 128 * kc : 128 * (kc + 1)],
                            rhs=qT[:, h, off:CHUNK],
                            start=True,
                            stop=True,
                        )
                        e = ep.tile([128, CHUNK], BF16, tag="e", bufs=8)
                        nc.scalar.activation(
                            out=e[:, off:CHUNK], in_=pss[:, off:CHUNK],
                            func=Act.Exp, scale=SCALE,
                        )
                        if t >= 0:
                            # triangular mask on the diagonal 128x128 block
                            nc.gpsimd.affine_select(
                                out=e[:, off : off + 128],
                                in_=e[:, off : off + 128],
                                compare_op=Alu.is_ge,
                                fill=0.0,
                                base=0,
                                channel_multiplier=-1,
                                pattern=[[1, 128]],
                            )
                        es.append((e, off))
                    return es

                def av_block(pso, es, kcs):
                    for (e, off), kc in zip(es, kcs):
                        nc.tensor.matmul(
                            pso[:, off:CHUNK],
                            lhsT=v_bf[:, kc, :],
                            rhs=e[:, off:CHUNK],
                            start=(kc == 0),
                            stop=(kc == nk - 1),
                            skip_group_check=True,
                        )

                def racc_block(racc, es, kcs):
                    for (e, off), kc in zip(es, kcs):
                        if kc == 0:
                            nc.vector.tensor_copy(out=racc[:], in_=e[:])
                        else:
                            nc.vector.tensor_tensor(
                                out=racc[:, off:CHUNK], in0=racc[:, off:CHUNK],
                                in1=e[:, off:CHUNK], op=Alu.add,
                            )

                pending = None
                for hp in range(NH // 2):
                    h0, h1 = 2 * hp, 2 * hp + 1
                    pso0 = ps_acc.tile([128, CHUNK], F32, tag="acc")
                    pso1 = ps_acc.tile([128, CHUNK], F32, tag="acc")
                    racc0 = ep.tile([128, CHUNK], BF16, tag="racc", bufs=2)
                    racc1 = ep.tile([128, CHUNK], BF16, tag="racc2", bufs=2)
                    first = True
                    for kb in range(0, nk, 4):
                        kcs = list(range(kb, min(kb + 4, nk)))
                        es0 = score_block(h0, kcs)
                        es1 = score_block(h1, kcs)
                        av_block(pso0, es0, kcs)
                        av_block(pso1, es1, kcs)
                        racc_block(racc0, es0, kcs)
                        racc_block(racc1, es1, kcs)
                        if first and pending is not None:
                            # pair0's epilogue lands after pair1's first
                            # kb-block so the racc tail is off the PE path
                            emit_epilogues(*pending)
                            pending = None
                        first = False
                    if pending is not None:
                        emit_epilogues(*pending)  # nk == 4: no second block
                    pending = (j, [(h0, pso0, racc0), (h1, pso1, racc1)])
                return pending

            def oproj(j, pend):
                # per-head gathered tiles: oth[h][:, r, :] = head 4r+h;
                # h-major accumulation consumes early AllGathers first.
                # The current chunk's deferred epilogue resolves behind
                # ss0's matmul stream (before ss2 needs its pso slots).
                oths = []
                for h in range(NH):
                    oth = otp.tile([128, NCORES, CHUNK], BF16, tag="ot")
                    nc.scalar.dma_start(
                        out=oth[:],
                        in_=attn_all[j][h][:].rearrange(
                            "(r p) s -> p r s", p=128
                        ),
                    )
                    oths.append(oth)
                for ss in range(CHUNK // 128):
                    qw = bass.ds(128 * ss, 128)
                    pso2 = ps_acc.tile([128, CHUNK], F32, tag="acc")
                    n = 0
                    for h in range(NH):
                        for r in range(NCORES):
                            nc.tensor.matmul(
                                pso2[:],
                                lhsT=oths[h][:, r, qw],
                                rhs=wo_bf[:, 4 * r + h, :],
                                start=(n == 0),
                                stop=(n == DC - 1),
                            )
                            n += 1
                    osb = ep.tile([128, CHUNK], F32, tag="os", bufs=3)
                    nc.scalar.copy(out=osb[:], in_=pso2[:])
                    r0 = j * CHUNK + 128 * ss
                    nc.sync.dma_start(out=out_d[r0 : r0 + 128, :], in_=osb[:])
                    if ss == 0 and pend is not None:
                        emit_epilogues(*pend)
                        pend = None

            # ---------------- schedule ----------------
            load_weight(wk_bf, wk_v, n=8)
            load_weight(wv_bf, wv_v, n=8)
            load_weight(wq_bf, wq_v)

            qT0 = qtp.tile([HD, NH, CHUNK], BF16, tag="qT")
            projections(0, qT0, None)
            pend = attention(0, qT0)
            load_weight(wo_bf, wo_v)

            for j in range(1, NCHUNK):
                qT = qtp.tile([HD, NH, CHUNK], BF16, tag="qT")
                # A(0)'s deferred pair lands in P(1); later ones land in
                # the previous chunk's oproj sandwich
                projections(j, qT, pend if j == 1 else None)
                new_pend = attention(j, qT)
                oproj(j - 1, new_pend)
            oproj(NCHUNK - 1, None)

    nc.finalize()
    return nc


def _get_graph():
    if "nc" not in _CACHED:
        _CACHED["nc"] = _build_graph()
    return _CACHED["nc"]


def _rope_tables(position_ids):
    pos = np.asarray(position_ids).reshape(-1).astype(np.float64)  # [S]
    inv_freq = 1.0 / (10000.0 ** (np.arange(0, HD, 2, dtype=np.float64) / HD))
    freqs = pos[:, None] * inv_freq[None, :]  # [S, 64]
    emb = np.concatenate([freqs, freqs], axis=-1)  # [S, HD]
    cos_t = np.cos(emb).T.astype(np.float32)  # [HD, S]
    sin_t = np.sin(emb).T.astype(np.float32)
    sin_signed = sin_t.copy()
    sin_signed[: HD // 2] *= -1.0
    bf = ml_dtypes.bfloat16
    return (
        np.ascontiguousarray(cos_t.astype(bf)),
        np.ascontiguousarray(sin_signed.astype(bf)),
    )


def kernel(hidden_states, wq, wk, wv, wo, position_ids, _trace=False):
    bf = ml_dtypes.bfloat16
    hs = np.asarray(hidden_states, np.float32).reshape(S, D)
    hsT = np.ascontiguousarray(hs.T.astype(bf))
    wq = np.asarray(wq, np.float32).astype(bf)
    wk = np.asarray(wk, np.float32).astype(bf)
    wv = np.asarray(wv, np.float32).astype(bf)
    wo = np.asarray(wo, np.float32).astype(bf)
    cos_t, sin_t = _rope_tables(position_ids)

    in_maps = []
    for c in range(NCORES):
        in_maps.append(
            {
                "hsT": hsT,
                "wq": np.ascontiguousarray(wq[:, QCOLS * c : QCOLS * (c + 1)]),
                "wk": np.ascontiguousarray(wk[:, HD * c : HD * (c + 1)]),
                "wv": np.ascontiguousarray(wv[:, HD * c : HD * (c + 1)]),
                "wo": np.ascontiguousarray(wo[:, QCOLS * c : QCOLS * (c + 1)]),
                "cos": cos_t,
                "sin": sin_t,
            }
        )

    nc = _get_graph()
    res = run_bass_kernel_spmd(
        nc, in_maps, core_ids=list(range(NCORES)), trace=_trace
    )
    outs = [np.asarray(res.results[c]["out"]) for c in range(NCORES)]
    full = np.concatenate(outs, axis=1).reshape(1, S, D).astype(np.float32)
    if _trace:
        kernel.last_results = res
    return full
